# revision 10
# baseline (speedup 1.0000x reference)
"""Trainium2 Bass kernel for nn_BoundedSoftmax (B=16, C=1024) on 8 NeuronCores.

Data-parallel over the batch: core c handles batch rows [2c, 2c+1].

Math (validated against the reference to ~2.5e-6 scale-relative):
  w = u - l;  s[i,j] = w[i] + w[j] + EPS  (symmetric)
  r = 1/s
  exp_l[i,j] = E_l[j]*F_u[i],  exp_u[i,j] = E_u[j]*F_l[i]   (separable)
  a_u*m_u = (E_u[j]*G1[i] - E_l[j]*G2[i]) * r,  G1 = m_u*F_l, G2 = m_u*F_u
  final_upper_coef = that, with diagonal replaced by -m_u*sum_a_u
  final_lower_coef = p_low[i]*E_m[j] off-diag (rank-1), diag -m_l*sum_a_l
  Row sums over j of a_u and a_u*l[j] are needed only for the upper bias and
  diagonal; since r is symmetric, sum_j V[j]*r[i,j] = sum_j V[j]*r[j,i], so
  TensorE computes all four weighted sums by contracting r tiles (partition
  dim = j-chunk) against V weights, accumulating in PSUM.

Device per tile (128 rows x 1024 cols):
  ACT:  s = Identity(w_bcast, bias=w[i]+EPS)
  DVE:  r = reciprocal_approx_fast(s)
  PE :  num = [G1; -G2]^T @ [E_u; E_l]  (K=2, fp32)    -> PSUM
        acc += V_I^T @ r                 (K=128, fp32)  -> PSUM (4,1024)
  DVE:  upper = num * r                                 -> SBUF (held)
  ACT:  lower = Copy(Em_bcast, scale=p_low[i])
  POOL: lower diag blend: blk = eye*delta_l + blk
  after the row's 8 tiles: finalize (4,1024) sums -> (128,8), compute
  upper bias + diagonal delta, blend upper diagonals, DMA everything out.

All (C,)-sized precomputes (exps, sums of exps, m_u/c_u/m_l/c_l, S_l/S_u,
biases of the separable lower path) are done on the host in fp32 with the
exact reference semantics; soft_lower/soft_upper/final_lower_bias are fully
separable elementwise O(B*C) and are returned from the host path.
"""

import numpy as np

EPS = np.float32(1e-12)
TINY = 1e-9
B, C = 16, 1024
N_CORES = 8
ROWS = B // N_CORES          # batch rows per core
NT = C // 128                # 128-row tiles per batch row

_CACHE = {}


def _build_device_kernel():
    import concourse.bass as bass
    import concourse.tile as tile
    from concourse.tile import add_dep_helper
    from concourse import bacc, mybir

    f32 = mybir.dt.float32
    Alu = mybir.AluOpType
    Act = mybir.ActivationFunctionType

    nc = bacc.Bacc("TRN2", target_bir_lowering=False, debug=False, num_devices=1)

    f32r = mybir.dt.float32r
    # num via 3-way float32r splits: 12 (lhs_row, rhs_row) product pairs
    g12_d = nc.dram_tensor("g12", [ROWS, 12, C], f32r, kind="ExternalInput")
    rhs_d = nc.dram_tensor("rhs_v", [ROWS, 12, C], f32r, kind="ExternalInput")
    vw_d = nc.dram_tensor("vw", [ROWS, 128, 4 * NT], f32r, kind="ExternalInput")
    wrow_d = nc.dram_tensor("wrow", [ROWS, 1, C], f32, kind="ExternalInput")
    emrow_d = nc.dram_tensor("emrow", [ROWS, 1, C], f32, kind="ExternalInput")
    # vecs columns (each NT wide): 0 w+EPS, 1 p_low, 2 delta_l, 3 G1, 4 G2,
    # 5 d_au, 6 d_au*l, 7 hvec, 8 u
    vecs_d = nc.dram_tensor("vecs", [ROWS, 128, 9 * NT], f32, kind="ExternalInput")

    up_d = nc.dram_tensor("out_up", [ROWS, C, C], f32, kind="ExternalOutput")
    lo_d = nc.dram_tensor("out_lo", [ROWS, C, C], f32, kind="ExternalOutput")
    ub_d = nc.dram_tensor("out_ubias", [ROWS, C], f32, kind="ExternalOutput")

    with tile.TileContext(nc) as tc:
        with (
            tc.tile_pool(name="smalls", bufs=2) as smalls,
            tc.tile_pool(name="bcast", bufs=2) as bcast,
            tc.tile_pool(name="stile", bufs=2) as sp,
            tc.tile_pool(name="rtile", bufs=3) as rp,
            tc.tile_pool(name="rftile", bufs=4) as rfp,
            tc.tile_pool(name="uptile", bufs=4) as upp,
            tc.tile_pool(name="lotile", bufs=3) as lop,
            tc.tile_pool(name="fins", bufs=2) as finp,
            tc.tile_pool(name="fint", bufs=10) as fint,
            tc.tile_pool(name="pnum", bufs=2, space="PSUM") as pnum,
            tc.tile_pool(name="pacc", bufs=2, space="PSUM") as pacc,
            tc.tile_pool(name="dscratch", bufs=2, space="DRAM") as dscr,
        ):
            for row in range(ROWS):
                vecs = smalls.tile([128, 9 * NT], f32, tag="vecs")
                nc.sync.dma_start(vecs[:], vecs_d[row])
                vw = smalls.tile([128, 4 * NT], f32r, tag="vw")
                nc.sync.dma_start(vw[:], vw_d[row])
                g12 = smalls.tile([12, C], f32r, tag="g12")
                nc.sync.dma_start(g12[:], g12_d[row])
                rhs = smalls.tile([12, C], f32r, tag="rhs")
                nc.sync.dma_start(rhs[:], rhs_d[row])

                w_b = bcast.tile([128, C], f32, tag="w_b")
                nc.sync.dma_start(w_b[:], wrow_d[row].partition_broadcast(128))
                em_b = bcast.tile([128, C], f32, tag="em_b")
                nc.sync.dma_start(em_b[:], emrow_d[row].partition_broadcast(128))

                def col(k, i0=0, n=NT):
                    return vecs[:, k * NT + i0 : k * NT + i0 + n]

                acc = pacc.tile([4, C], f32)
                up_dmas = []
                lo_dmas = []
                for I in range(NT):
                    blk = slice(I * 128, (I + 1) * 128)
                    s = sp.tile([128, C], f32, tag="s")
                    nc.scalar.activation(s[:], w_b[:], Act.Identity,
                                         bias=col(0, I, 1), scale=1.0)
                    r = rp.tile([128, C], f32, tag="r")
                    nc.vector.reciprocal_approx_fast(r[:], s[:])
                    rf = rfp.tile([128, C], f32r, tag="rf")
                    nc.gpsimd.dma_start(rf[:], r[:])

                    # weighted row sums: acc[m, :] += sum_p V[p, m] * rf[p, :]
                    nc.tensor.matmul(acc[:, 0:512], lhsT=vw[:, 4 * I : 4 * I + 4],
                                     rhs=rf[:, 0:512], start=(I == 0), stop=(I == NT - 1))
                    nc.tensor.matmul(acc[:, 512:1024], lhsT=vw[:, 4 * I : 4 * I + 4],
                                     rhs=rf[:, 512:1024], start=(I == 0), stop=(I == NT - 1))

                    num = pnum.tile([128, C], f32, tag="num")
                    nc.tensor.matmul(num[:, 0:512], lhsT=g12[:, blk],
                                     rhs=rhs[:, 0:512], start=True, stop=True)
                    nc.tensor.matmul(num[:, 512:1024], lhsT=g12[:, blk],
                                     rhs=rhs[:, 512:1024], start=True, stop=True)

                    up = upp.tile([128, C], f32, tag="up")
                    nc.vector.tensor_mul(up[:], num[:], r[:])
                    up_dmas.append(nc.sync.dma_start(up_d[row, blk, :], up[:]))

                    lo = lop.tile([128, C], f32, tag="lo")
                    nc.scalar.mul(lo[:], em_b[:], col(1, I, 1))
                    lo_dmas.append(nc.sync.dma_start(lo_d[row, blk, :], lo[:]))

                # ---- finalize row ----
                acc_sb = finp.tile([4, C], f32, tag="acc_sb")
                nc.scalar.copy(acc_sb[:], acc[:])
                scr = dscr.tile([4, C], f32, tag="scr")
                nc.sync.dma_start(scr[:], acc_sb[:])
                racc = finp.tile([128, 4 * NT], f32, tag="racc")
                # racc[p, m*NT+t] = scr[m, t*128+p]
                nc.sync.dma_start(
                    racc[:], scr[:].rearrange("m (t p) -> p (m t)", p=128))

                def R(m):
                    return racc[:, m * NT : (m + 1) * NT]

                def t_(nm):
                    return fint.tile([128, NT], f32, tag="ft", name=nm)
                m1 = t_("m1"); nc.vector.tensor_mul(m1[:], R(0), col(3))
                m2 = t_("m2"); nc.vector.tensor_mul(m2[:], R(1), col(4))
                s1 = t_("s1"); nc.vector.tensor_sub(s1[:], m1[:], m2[:])
                sum_s = t_("sum_s"); nc.vector.tensor_sub(sum_s[:], s1[:], col(5))
                m3 = t_("m3"); nc.vector.tensor_mul(m3[:], R(2), col(3))
                m4 = t_("m4"); nc.vector.tensor_mul(m4[:], R(3), col(4))
                s2 = t_("s2"); nc.vector.tensor_sub(s2[:], m3[:], m4[:])
                W = t_("W"); nc.vector.tensor_sub(W[:], s2[:], col(6))
                t6 = t_("t6"); nc.vector.tensor_mul(t6[:], col(8), sum_s[:])
                t7 = t_("t7"); nc.vector.tensor_sub(t7[:], col(7), W[:])
                ub = fint.tile([128, NT], f32, tag="ub")
                nc.vector.tensor_add(ub[:], t7[:], t6[:])
                du = fint.tile([128, NT], f32, tag="du")
                nc.vector.tensor_sub(du[:], col(5), s1[:])

                nc.sync.dma_start(
                    ub_d[row].rearrange("(t p) -> p t", p=128), ub[:])

                # diagonal scatters, ordered after the full-tile writes
                updiag = up_d[row].rearrange("a b -> (a b)")[0 : C * C : C + 1]
                updiag = updiag.rearrange("(t p) -> p t", p=128)
                dd1 = nc.sync.dma_start(updiag, du[:])
                lodiag = lo_d[row].rearrange("a b -> (a b)")[0 : C * C : C + 1]
                lodiag = lodiag.rearrange("(t p) -> p t", p=128)
                dd2 = nc.sync.dma_start(lodiag, col(2))
                for dmas, dd in ((up_dmas, dd1), (lo_dmas, dd2)):
                    for dm in dmas:
                        add_dep_helper(dd.ins, dm.ins, True,
                                       "diag scatter after tile write")

    nc.finalize()
    return nc


def _host_precompute(lower, upper):
    """All (C,)-sized vectors, fp32, reference semantics. Returns per-B dict."""
    f = np.float32
    l = lower.astype(f); u = upper.astype(f)
    w = u - l
    E_l = np.exp(l); E_u = np.exp(u); F_l = np.exp(-l); F_u = np.exp(-u)
    m = (f(0.5) * (l + u)).astype(f)
    E_m = np.exp(m).astype(f); F_m = np.exp(-m).astype(f)
    T_l = E_l.sum(1, keepdims=True).astype(f)
    T_u = E_u.sum(1, keepdims=True).astype(f)
    T_m = E_m.sum(1, keepdims=True).astype(f)
    T_mm = (E_m * m).sum(1, keepdims=True).astype(f)

    S_l = (F_u * (T_l - E_l)).astype(f)
    S_u = (F_l * (T_u - E_u)).astype(f)
    g_l = (f(1) / (f(1) + S_l)).astype(f)
    g_u = (f(1) / (f(1) + S_u)).astype(f)
    denom_g = (S_u - S_l).astype(f)
    m_u = ((g_u - g_l) / (denom_g + EPS)).astype(f)
    m_u = np.where(np.abs(denom_g) < TINY, -f(1) / (f(1) + S_u) ** 2, m_u).astype(f)
    c_u = (g_l - m_u * S_l).astype(f)
    m_l = (-f(1) / (f(1) + S_u) ** 2).astype(f)
    c_l = (g_u - m_l * S_u).astype(f)

    soft_lower = np.clip(g_u, 0.0, 1.0).astype(f)
    soft_upper = np.clip(g_l, 0.0, 1.0).astype(f)

    # separable lower path
    sum_a_l = (F_m * (T_m - E_m)).astype(f)
    sum_alt = (F_m * (T_mm - E_m * m)).astype(f)
    bias_S_lower = (sum_a_l - (sum_alt - m * sum_a_l)).astype(f)
    final_lower_bias = (m_l * bias_S_lower + c_l).astype(f)
    p_low = (m_l * F_m).astype(f)
    D_l = (-m_l * sum_a_l).astype(f)

    # upper path device inputs
    G1 = (m_u * F_l).astype(f)
    G2 = (m_u * F_u).astype(f)
    wEPS = (w + EPS).astype(f)
    r_diag = (f(1) / (2 * w + EPS)).astype(f)
    d_au = ((E_u * G1 - E_l * G2) * r_diag).astype(f)
    d_au_l = (d_au * l).astype(f)
    hvec = (m_u * S_l + c_u).astype(f)

    return dict(l=l, u=u, w=w, E_l=E_l, E_u=E_u, E_m=E_m, m_u=m_u,
                soft_lower=soft_lower, soft_upper=soft_upper,
                final_lower_bias=final_lower_bias,
                p_low=p_low, D_l=D_l, G1=G1, G2=G2, wEPS=wEPS,
                d_au=d_au, d_au_l=d_au_l, hvec=hvec)


def _colpack(v):
    """(C,) -> (128, NT) with value for i = t*128+p at [p, t]."""
    return np.ascontiguousarray(v.reshape(NT, 128).T)


def _f32r(x):
    """Round fp32 to the float32r grid (RNE at 11 explicit mantissa bits)."""
    xi = x.view(np.int32).astype(np.int64)
    drop = 12
    half = 1 << (drop - 1)
    mask = (1 << drop) - 1
    rn = (xi + half) & ~mask
    tie = (xi & mask) == half
    out = np.where(tie & (((xi >> drop) & 1) == 0), xi & ~mask, rn)
    return out.astype(np.int64).astype(np.int32).view(np.float32).astype(np.float32)


def _split3(x):
    """x -> (hi, mid, lo) on the f32r grid with hi+mid+lo ~= x to ~2^-36."""
    hi = _f32r(x)
    mid = _f32r((x - hi).astype(np.float32))
    lo = _f32r((x - hi - mid).astype(np.float32))
    return hi, mid, lo


def _num_split_rows(a, b):
    """Product-pair rows for a*b via 3-way splits: (lhs_rows, rhs_rows),
    6 rows each, representing a*b to ~2^-35."""
    ah, am, al = _split3(a)
    bh, bm, bl = _split3(b)
    lhs = [ah, ah, am, ah, al, am]
    rhs = [bh, bm, bh, bl, bh, bm]
    return lhs, rhs


def kernel(lower, upper):
    from concourse.bass_utils import run_bass_kernel_spmd

    lower = np.asarray(lower); upper = np.asarray(upper)
    assert lower.shape == (B, C) and upper.shape == (B, C)
    h = _host_precompute(lower, upper)

    in_maps = []
    for c in range(N_CORES):
        rows = [ROWS * c + r for r in range(ROWS)]
        g12_rows = []
        rhs_rows = []
        for b in rows:
            l1, r1 = _num_split_rows(h["G1"][b], h["E_u"][b])
            l2, r2 = _num_split_rows((-h["G2"][b]).astype(np.float32), h["E_l"][b])
            g12_rows.append(np.stack(l1 + l2))
            rhs_rows.append(np.stack(r1 + r2))
        g12 = np.stack(g12_rows)
        rhs_v = np.stack(rhs_rows)
        vw = np.stack([
            _f32r(np.ascontiguousarray(
                np.stack([h["E_u"][b], h["E_l"][b],
                          h["E_u"][b] * h["l"][b], h["E_l"][b] * h["l"][b]],
                         axis=1).reshape(NT, 128, 4).transpose(1, 0, 2)
                .reshape(128, 4 * NT)))
            for b in rows])
        wrow = np.stack([h["w"][b][None, :] for b in rows])
        emrow = np.stack([h["E_m"][b][None, :] for b in rows])
        vecs = np.stack([
            np.concatenate([
                _colpack(h["wEPS"][b]), _colpack(h["p_low"][b]),
                _colpack(h["D_l"][b]), _colpack(h["G1"][b]),
                _colpack(h["G2"][b]), _colpack(h["d_au"][b]),
                _colpack(h["d_au_l"][b]), _colpack(h["hvec"][b]),
                _colpack(h["u"][b])], axis=1)
            for b in rows])
        in_maps.append({
            "g12": np.ascontiguousarray(g12, np.float32),
            "rhs_v": np.ascontiguousarray(rhs_v, np.float32),
            "vw": np.ascontiguousarray(vw, np.float32),
            "wrow": np.ascontiguousarray(wrow, np.float32),
            "emrow": np.ascontiguousarray(emrow, np.float32),
            "vecs": np.ascontiguousarray(vecs, np.float32),
        })

    if "nc" not in _CACHE:
        _CACHE["nc"] = _build_device_kernel()
    nc = _CACHE["nc"]

    res = run_bass_kernel_spmd(nc, in_maps, list(range(N_CORES)),
                               **_CACHE.get("run_kwargs", {}))
    _CACHE["last_results"] = res

    final_upper_coef = np.empty((B, C, C), np.float32)
    final_lower_coef = np.empty((B, C, C), np.float32)
    final_upper_bias = np.empty((B, C), np.float32)
    for c in range(N_CORES):
        out = res.results[c]
        final_upper_coef[ROWS * c : ROWS * (c + 1)] = out["out_up"]
        final_lower_coef[ROWS * c : ROWS * (c + 1)] = out["out_lo"]
        final_upper_bias[ROWS * c : ROWS * (c + 1)] = out["out_ubias"]

    return (h["soft_lower"], h["soft_upper"],
            final_lower_coef, final_upper_coef,
            h["final_lower_bias"], final_upper_bias)


# revision 11
# speedup vs baseline: 1.2363x; 1.2363x over previous
"""Trainium2 Bass kernel for nn_BoundedSoftmax (B=16, C=1024) on 8 NeuronCores.

Data-parallel over the batch: core c handles batch rows [2c, 2c+1].

Math (validated against the reference to ~2.5e-6 scale-relative):
  w = u - l;  s[i,j] = w[i] + w[j] + EPS  (symmetric)
  r = 1/s
  exp_l[i,j] = E_l[j]*F_u[i],  exp_u[i,j] = E_u[j]*F_l[i]   (separable)
  a_u*m_u = (E_u[j]*G1[i] - E_l[j]*G2[i]) * r,  G1 = m_u*F_l, G2 = m_u*F_u
  final_upper_coef = that, with diagonal replaced by -m_u*sum_a_u
  final_lower_coef = p_low[i]*E_m[j] off-diag (rank-1), diag -m_l*sum_a_l
  Row sums over j of a_u and a_u*l[j] are needed only for the upper bias and
  diagonal; since r is symmetric, sum_j V[j]*r[i,j] = sum_j V[j]*r[j,i], so
  TensorE computes all four weighted sums by contracting r tiles (partition
  dim = j-chunk) against V weights, accumulating in PSUM.

Device per tile (128 rows x 1024 cols):
  ACT:  s = Identity(w_bcast, bias=w[i]+EPS)
  DVE:  r = reciprocal_approx_fast(s)
  PE :  num = [G1; -G2]^T @ [E_u; E_l]  (K=2, fp32)    -> PSUM
        acc += V_I^T @ r                 (K=128, fp32)  -> PSUM (4,1024)
  DVE:  upper = num * r                                 -> SBUF (held)
  ACT:  lower = Copy(Em_bcast, scale=p_low[i])
  POOL: lower diag blend: blk = eye*delta_l + blk
  after the row's 8 tiles: finalize (4,1024) sums -> (128,8), compute
  upper bias + diagonal delta, blend upper diagonals, DMA everything out.

All (C,)-sized precomputes (exps, sums of exps, m_u/c_u/m_l/c_l, S_l/S_u,
biases of the separable lower path) are done on the host in fp32 with the
exact reference semantics; soft_lower/soft_upper/final_lower_bias are fully
separable elementwise O(B*C) and are returned from the host path.
"""

import numpy as np

EPS = np.float32(1e-12)
TINY = 1e-9
B, C = 16, 1024
N_CORES = 8
ROWS = B // N_CORES          # batch rows per core
NT = C // 128                # 128-row tiles per batch row

_CACHE = {}


def _build_device_kernel():
    import concourse.bass as bass
    import concourse.tile as tile
    from concourse.tile import add_dep_helper
    from concourse import bacc, mybir

    f32 = mybir.dt.float32
    Alu = mybir.AluOpType
    Act = mybir.ActivationFunctionType

    nc = bacc.Bacc("TRN2", target_bir_lowering=False, debug=False, num_devices=1)

    f32r = mybir.dt.float32r
    # num via 3-way float32r splits: 12 (lhs_row, rhs_row) product pairs
    g12_d = nc.dram_tensor("g12", [ROWS, 12, C], f32r, kind="ExternalInput")
    rhs_d = nc.dram_tensor("rhs_v", [ROWS, 12, C], f32r, kind="ExternalInput")
    vw_d = nc.dram_tensor("vw", [ROWS, 128, 4 * NT], f32r, kind="ExternalInput")
    wrow_d = nc.dram_tensor("wrow", [ROWS, 1, C], f32, kind="ExternalInput")
    emrow_d = nc.dram_tensor("emrow", [ROWS, 1, C], f32, kind="ExternalInput")
    # vecs columns (each NT wide): 0 w+EPS, 1 p_low, 2 delta_l, 3 G1, 4 G2,
    # 5 d_au, 6 d_au*l, 7 hvec, 8 u
    vecs_d = nc.dram_tensor("vecs", [ROWS, 128, 9 * NT], f32, kind="ExternalInput")

    up_d = nc.dram_tensor("out_up", [ROWS, C, C], f32, kind="ExternalOutput")
    lo_d = nc.dram_tensor("out_lo", [ROWS, C, C], f32, kind="ExternalOutput")
    ub_d = nc.dram_tensor("out_ubias", [ROWS, C], f32, kind="ExternalOutput")

    with tile.TileContext(nc) as tc:
        with (
            tc.tile_pool(name="const", bufs=1) as constp,
            tc.tile_pool(name="smalls", bufs=2) as smalls,
            tc.tile_pool(name="bcast", bufs=2) as bcast,
            tc.tile_pool(name="stile", bufs=4) as sp,
            tc.tile_pool(name="rtile", bufs=4) as rp,
            tc.tile_pool(name="rftile", bufs=4) as rfp,
            tc.tile_pool(name="uptile", bufs=4) as upp,
            tc.tile_pool(name="lotile", bufs=4) as lop,
            tc.tile_pool(name="fins", bufs=2) as finp,
            tc.tile_pool(name="fint", bufs=10) as fint,
            tc.tile_pool(name="pnum", bufs=2, space="PSUM") as pnum,
            tc.tile_pool(name="pacc", bufs=1, space="PSUM") as pacc,
            tc.tile_pool(name="pbcast", bufs=1, space="PSUM") as pbc,
            tc.tile_pool(name="dscratch", bufs=2, space="DRAM") as dscr,
        ):
            ones = constp.tile([1, 128], f32)
            nc.vector.memset(ones[:], 1.0)
            for row in range(ROWS):
                vecs = smalls.tile([128, 9 * NT], f32, tag="vecs")
                nc.sync.dma_start(vecs[:], vecs_d[row])
                vw = smalls.tile([128, 4 * NT], f32r, tag="vw")
                nc.sync.dma_start(vw[:], vw_d[row])
                g12 = smalls.tile([12, C], f32r, tag="g12")
                nc.sync.dma_start(g12[:], g12_d[row])
                rhs = smalls.tile([12, C], f32r, tag="rhs")
                nc.sync.dma_start(rhs[:], rhs_d[row])

                wrow = smalls.tile([1, C], f32, tag="wrow")
                nc.sync.dma_start(wrow[:], wrow_d[row])
                emrow = smalls.tile([1, C], f32, tag="emrow")
                nc.sync.dma_start(emrow[:], emrow_d[row])
                w_b = bcast.tile([128, C], f32, tag="w_b")
                em_b = bcast.tile([128, C], f32, tag="em_b")
                for src_row, dst in ((wrow, w_b), (emrow, em_b)):
                    pb = pbc.tile([128, C], f32, tag="pb")
                    nc.tensor.matmul(pb[:, 0:512], lhsT=ones[:], rhs=src_row[:, 0:512],
                                     start=True, stop=True)
                    nc.tensor.matmul(pb[:, 512:1024], lhsT=ones[:], rhs=src_row[:, 512:1024],
                                     start=True, stop=True)
                    nc.scalar.copy(dst[:], pb[:])

                def col(k, i0=0, n=NT):
                    return vecs[:, k * NT + i0 : k * NT + i0 + n]

                acc = pacc.tile([4, C], f32)
                up_dmas = []
                lo_dmas = []
                for I in range(NT):
                    blk = slice(I * 128, (I + 1) * 128)
                    s = sp.tile([128, C], f32, tag="s")
                    nc.scalar.activation(s[:], w_b[:], Act.Identity,
                                         bias=col(0, I, 1), scale=1.0)
                    r = rp.tile([128, C], f32, tag="r")
                    nc.vector.reciprocal_approx_fast(r[:], s[:])
                    rf = rfp.tile([128, C], f32r, tag="rf")
                    nc.vector.tensor_copy(rf[:], r[:])

                    # weighted row sums: acc[m, :] += sum_p V[p, m] * rf[p, :]
                    nc.tensor.matmul(acc[:, 0:512], lhsT=vw[:, 4 * I : 4 * I + 4],
                                     rhs=rf[:, 0:512], start=(I == 0), stop=(I == NT - 1))
                    nc.tensor.matmul(acc[:, 512:1024], lhsT=vw[:, 4 * I : 4 * I + 4],
                                     rhs=rf[:, 512:1024], start=(I == 0), stop=(I == NT - 1))

                    num = pnum.tile([128, C], f32, tag="num")
                    nc.tensor.matmul(num[:, 0:512], lhsT=g12[:, blk],
                                     rhs=rhs[:, 0:512], start=True, stop=True)
                    nc.tensor.matmul(num[:, 512:1024], lhsT=g12[:, blk],
                                     rhs=rhs[:, 512:1024], start=True, stop=True)

                    up = upp.tile([128, C], f32, tag="up")
                    nc.vector.tensor_mul(up[:], num[:], r[:])
                    up_dmas.append(nc.sync.dma_start(up_d[row, blk, :], up[:]))

                    lo = lop.tile([128, C], f32, tag="lo")
                    nc.scalar.mul(lo[:], em_b[:], col(1, I, 1))
                    lo_dmas.append(nc.sync.dma_start(lo_d[row, blk, :], lo[:]))

                # ---- finalize row ----
                acc_sb = finp.tile([4, C], f32, tag="acc_sb")
                nc.scalar.copy(acc_sb[:], acc[:])
                scr = dscr.tile([4, C], f32, tag="scr")
                nc.sync.dma_start(scr[:], acc_sb[:])
                racc = finp.tile([128, 4 * NT], f32, tag="racc")
                # racc[p, m*NT+t] = scr[m, t*128+p]
                nc.sync.dma_start(
                    racc[:], scr[:].rearrange("m (t p) -> p (m t)", p=128))

                def R(m):
                    return racc[:, m * NT : (m + 1) * NT]

                def t_(nm):
                    return fint.tile([128, NT], f32, tag="ft", name=nm)
                m1 = t_("m1"); nc.vector.tensor_mul(m1[:], R(0), col(3))
                m2 = t_("m2"); nc.vector.tensor_mul(m2[:], R(1), col(4))
                s1 = t_("s1"); nc.vector.tensor_sub(s1[:], m1[:], m2[:])
                sum_s = t_("sum_s"); nc.vector.tensor_sub(sum_s[:], s1[:], col(5))
                m3 = t_("m3"); nc.vector.tensor_mul(m3[:], R(2), col(3))
                m4 = t_("m4"); nc.vector.tensor_mul(m4[:], R(3), col(4))
                s2 = t_("s2"); nc.vector.tensor_sub(s2[:], m3[:], m4[:])
                W = t_("W"); nc.vector.tensor_sub(W[:], s2[:], col(6))
                t6 = t_("t6"); nc.vector.tensor_mul(t6[:], col(8), sum_s[:])
                t7 = t_("t7"); nc.vector.tensor_sub(t7[:], col(7), W[:])
                ub = fint.tile([128, NT], f32, tag="ub")
                nc.vector.tensor_add(ub[:], t7[:], t6[:])
                du = fint.tile([128, NT], f32, tag="du")
                nc.vector.tensor_sub(du[:], col(5), s1[:])

                nc.sync.dma_start(
                    ub_d[row].rearrange("(t p) -> p t", p=128), ub[:])

                # diagonal scatters, ordered after the full-tile writes
                updiag = up_d[row].rearrange("a b -> (a b)")[0 : C * C : C + 1]
                updiag = updiag.rearrange("(t p) -> p t", p=128)
                dd1 = nc.sync.dma_start(updiag, du[:])
                lodiag = lo_d[row].rearrange("a b -> (a b)")[0 : C * C : C + 1]
                lodiag = lodiag.rearrange("(t p) -> p t", p=128)
                dd2 = nc.sync.dma_start(lodiag, col(2))
                for dmas, dd in ((up_dmas, dd1), (lo_dmas, dd2)):
                    for dm in dmas:
                        add_dep_helper(dd.ins, dm.ins, True,
                                       "diag scatter after tile write")

    nc.finalize()
    return nc


def _host_precompute(lower, upper):
    """All (C,)-sized vectors, fp32, reference semantics. Returns per-B dict."""
    f = np.float32
    l = lower.astype(f); u = upper.astype(f)
    w = u - l
    E_l = np.exp(l); E_u = np.exp(u); F_l = np.exp(-l); F_u = np.exp(-u)
    m = (f(0.5) * (l + u)).astype(f)
    E_m = np.exp(m).astype(f); F_m = np.exp(-m).astype(f)
    T_l = E_l.sum(1, keepdims=True).astype(f)
    T_u = E_u.sum(1, keepdims=True).astype(f)
    T_m = E_m.sum(1, keepdims=True).astype(f)
    T_mm = (E_m * m).sum(1, keepdims=True).astype(f)

    S_l = (F_u * (T_l - E_l)).astype(f)
    S_u = (F_l * (T_u - E_u)).astype(f)
    g_l = (f(1) / (f(1) + S_l)).astype(f)
    g_u = (f(1) / (f(1) + S_u)).astype(f)
    denom_g = (S_u - S_l).astype(f)
    m_u = ((g_u - g_l) / (denom_g + EPS)).astype(f)
    m_u = np.where(np.abs(denom_g) < TINY, -f(1) / (f(1) + S_u) ** 2, m_u).astype(f)
    c_u = (g_l - m_u * S_l).astype(f)
    m_l = (-f(1) / (f(1) + S_u) ** 2).astype(f)
    c_l = (g_u - m_l * S_u).astype(f)

    soft_lower = np.clip(g_u, 0.0, 1.0).astype(f)
    soft_upper = np.clip(g_l, 0.0, 1.0).astype(f)

    # separable lower path
    sum_a_l = (F_m * (T_m - E_m)).astype(f)
    sum_alt = (F_m * (T_mm - E_m * m)).astype(f)
    bias_S_lower = (sum_a_l - (sum_alt - m * sum_a_l)).astype(f)
    final_lower_bias = (m_l * bias_S_lower + c_l).astype(f)
    p_low = (m_l * F_m).astype(f)
    D_l = (-m_l * sum_a_l).astype(f)

    # upper path device inputs
    G1 = (m_u * F_l).astype(f)
    G2 = (m_u * F_u).astype(f)
    wEPS = (w + EPS).astype(f)
    r_diag = (f(1) / (2 * w + EPS)).astype(f)
    d_au = ((E_u * G1 - E_l * G2) * r_diag).astype(f)
    d_au_l = (d_au * l).astype(f)
    hvec = (m_u * S_l + c_u).astype(f)

    return dict(l=l, u=u, w=w, E_l=E_l, E_u=E_u, E_m=E_m, m_u=m_u,
                soft_lower=soft_lower, soft_upper=soft_upper,
                final_lower_bias=final_lower_bias,
                p_low=p_low, D_l=D_l, G1=G1, G2=G2, wEPS=wEPS,
                d_au=d_au, d_au_l=d_au_l, hvec=hvec)


def _colpack(v):
    """(C,) -> (128, NT) with value for i = t*128+p at [p, t]."""
    return np.ascontiguousarray(v.reshape(NT, 128).T)


def _f32r(x):
    """Round fp32 to the float32r grid (RNE at 11 explicit mantissa bits)."""
    xi = x.view(np.int32).astype(np.int64)
    drop = 12
    half = 1 << (drop - 1)
    mask = (1 << drop) - 1
    rn = (xi + half) & ~mask
    tie = (xi & mask) == half
    out = np.where(tie & (((xi >> drop) & 1) == 0), xi & ~mask, rn)
    return out.astype(np.int64).astype(np.int32).view(np.float32).astype(np.float32)


def _split3(x):
    """x -> (hi, mid, lo) on the f32r grid with hi+mid+lo ~= x to ~2^-36."""
    hi = _f32r(x)
    mid = _f32r((x - hi).astype(np.float32))
    lo = _f32r((x - hi - mid).astype(np.float32))
    return hi, mid, lo


def _num_split_rows(a, b):
    """Product-pair rows for a*b via 3-way splits: (lhs_rows, rhs_rows),
    6 rows each, representing a*b to ~2^-35."""
    ah, am, al = _split3(a)
    bh, bm, bl = _split3(b)
    lhs = [ah, ah, am, ah, al, am]
    rhs = [bh, bm, bh, bl, bh, bm]
    return lhs, rhs


def kernel(lower, upper):
    from concourse.bass_utils import run_bass_kernel_spmd

    lower = np.asarray(lower); upper = np.asarray(upper)
    assert lower.shape == (B, C) and upper.shape == (B, C)
    h = _host_precompute(lower, upper)

    in_maps = []
    for c in range(N_CORES):
        rows = [ROWS * c + r for r in range(ROWS)]
        g12_rows = []
        rhs_rows = []
        for b in rows:
            l1, r1 = _num_split_rows(h["G1"][b], h["E_u"][b])
            l2, r2 = _num_split_rows((-h["G2"][b]).astype(np.float32), h["E_l"][b])
            g12_rows.append(np.stack(l1 + l2))
            rhs_rows.append(np.stack(r1 + r2))
        g12 = np.stack(g12_rows)
        rhs_v = np.stack(rhs_rows)
        vw = np.stack([
            _f32r(np.ascontiguousarray(
                np.stack([h["E_u"][b], h["E_l"][b],
                          h["E_u"][b] * h["l"][b], h["E_l"][b] * h["l"][b]],
                         axis=1).reshape(NT, 128, 4).transpose(1, 0, 2)
                .reshape(128, 4 * NT)))
            for b in rows])
        wrow = np.stack([h["w"][b][None, :] for b in rows])
        emrow = np.stack([h["E_m"][b][None, :] for b in rows])
        vecs = np.stack([
            np.concatenate([
                _colpack(h["wEPS"][b]), _colpack(h["p_low"][b]),
                _colpack(h["D_l"][b]), _colpack(h["G1"][b]),
                _colpack(h["G2"][b]), _colpack(h["d_au"][b]),
                _colpack(h["d_au_l"][b]), _colpack(h["hvec"][b]),
                _colpack(h["u"][b])], axis=1)
            for b in rows])
        in_maps.append({
            "g12": np.ascontiguousarray(g12, np.float32),
            "rhs_v": np.ascontiguousarray(rhs_v, np.float32),
            "vw": np.ascontiguousarray(vw, np.float32),
            "wrow": np.ascontiguousarray(wrow, np.float32),
            "emrow": np.ascontiguousarray(emrow, np.float32),
            "vecs": np.ascontiguousarray(vecs, np.float32),
        })

    if "nc" not in _CACHE:
        _CACHE["nc"] = _build_device_kernel()
    nc = _CACHE["nc"]

    res = run_bass_kernel_spmd(nc, in_maps, list(range(N_CORES)),
                               **_CACHE.get("run_kwargs", {}))
    _CACHE["last_results"] = res

    final_upper_coef = np.empty((B, C, C), np.float32)
    final_lower_coef = np.empty((B, C, C), np.float32)
    final_upper_bias = np.empty((B, C), np.float32)
    for c in range(N_CORES):
        out = res.results[c]
        final_upper_coef[ROWS * c : ROWS * (c + 1)] = out["out_up"]
        final_lower_coef[ROWS * c : ROWS * (c + 1)] = out["out_lo"]
        final_upper_bias[ROWS * c : ROWS * (c + 1)] = out["out_ubias"]

    return (h["soft_lower"], h["soft_upper"],
            final_lower_coef, final_upper_coef,
            h["final_lower_bias"], final_upper_bias)
